# revision 3
# baseline (speedup 1.0000x reference)
"""RotatE KGE scoring kernel for Trainium2 (Bass/Tile), 8-core data parallel.

Problem (per reference):
  head  = entity_embedding[head_part[:,0]]           # [B,1,1000]
  rel   = relation_embedding[head_part[:,1]]         # [B,1,500]
  tail  = entity_embedding[tail_part]                # [B,128,1000]
  phase = rel / (EMB_RANGE/PI); rot = head * e^{i*phase}  (complex, D/2=500)
  score = GAMMA - sum_d sqrt((rot_re-tail_re)^2 + (rot_im-tail_im)^2)

Sharding: batch dim (1024) split across 8 cores, 128 batches each; embedding
tables replicated. Per core the dominant cost is the tail gather:
128x128 rows x 4KB = 65.5 MB of HBM traffic (memory-bound regime).

Per-core device pipeline (j = neg-sample index, 128 iterations):
  DVE : copy -rot_cat into tile (f32 copy runs in 2x port mode)
  DMA : indirect row-gather of entity rows with compute_op=add
        -> tile becomes (tail - rot) directly, no subtract pass needed
  ACT : Square of re-half -> SBUF
  DVE : square of im-half (tensor_mul)
  PE  : identity-matmul accumulate re^2 + im^2 -> PSUM
  ACT : Sqrt with accum_out -> one column of the [128,128] score tile
Final: score = GAMMA - colsums, one activation + one DMA out.
"""

import math
from contextlib import ExitStack

import numpy as np

import concourse.bacc as bacc
import concourse.mybir as mybir
import concourse.tile as tile
from concourse.bass import IndirectOffsetOnAxis
from concourse.bass_utils import run_bass_kernel_spmd
from concourse.masks import make_identity

# ---- problem constants (hardcoded per contract) ----
N_CORES = 8
B = 1024
B_LOC = B // N_CORES  # 128
NEG = 128
N_ENT = 100000
N_REL = 500
D = 1000
D2 = D // 2  # 500

GAMMA = 12.0
EPSILON = 2.0
EMB_RANGE = (GAMMA + EPSILON) / D2  # 0.028
PI = 3.141592653589793
PHASE_SCALE = float(1.0 / (EMB_RANGE / PI))  # multiply instead of divide

TWO_PI = 2.0 * math.pi
INV_TWO_PI = 1.0 / TWO_PI
MAGIC = 1.5 * 2.0**23  # round-to-nearest via fp32 quantization
# Cody-Waite split of 2*pi: c0 exact in fp32, c1 fp32, c2 the f64 remainder
CW0 = 6.28125
CW1 = float(np.float32(TWO_PI - CW0))
CW2 = float(TWO_PI - CW0 - np.float64(np.float32(TWO_PI - CW0)))

f32 = mybir.dt.float32
i32 = mybir.dt.int32
AF = mybir.ActivationFunctionType
ALU = mybir.AluOpType

_CACHED_NC = None


def _build_nc():
    nc = bacc.Bacc("TRN2", target_bir_lowering=False, debug=False)

    hp = nc.dram_tensor("head_part", [B_LOC, 3], i32, kind="ExternalInput")
    tp = nc.dram_tensor("tail_part", [B_LOC, NEG], i32, kind="ExternalInput")
    rel = nc.dram_tensor("relation_embedding", [N_REL, D2], f32, kind="ExternalInput")
    ent = nc.dram_tensor("entity_embedding", [N_ENT, D], f32, kind="ExternalInput")
    score = nc.dram_tensor("score", [B_LOC, NEG], f32, kind="ExternalOutput")

    P = 128

    with tile.TileContext(nc) as tc, ExitStack() as ctx:
        const = ctx.enter_context(tc.tile_pool(name="const", bufs=1))
        pre = ctx.enter_context(tc.tile_pool(name="pre", bufs=1))
        tails = ctx.enter_context(tc.tile_pool(name="tails", bufs=6))
        sqp = ctx.enter_context(tc.tile_pool(name="sqp", bufs=4))
        psum = ctx.enter_context(tc.tile_pool(name="psum", bufs=4, space="PSUM"))
        psc = ctx.enter_context(tc.tile_pool(name="psc", bufs=2, space="PSUM"))

        # ---------- preamble ----------
        hp_t = const.tile([P, 3], i32)
        nc.sync.dma_start(out=hp_t[:], in_=hp[:])
        tp_t = const.tile([P, NEG], i32)
        nc.sync.dma_start(out=tp_t[:], in_=tp[:])

        head_t = pre.tile([P, D], f32)
        nc.gpsimd.indirect_dma_start(
            out=head_t[:], out_offset=None, in_=ent[:],
            in_offset=IndirectOffsetOnAxis(ap=hp_t[:, 0:1], axis=0),
        )
        relv = pre.tile([P, D2], f32)
        nc.gpsimd.indirect_dma_start(
            out=relv[:], out_offset=None, in_=rel[:],
            in_offset=IndirectOffsetOnAxis(ap=hp_t[:, 1:2], axis=0),
        )

        def const_col(val):
            t = const.tile([P, 1], f32, tag=f"c{val}")
            nc.gpsimd.memset(t[:], float(val))
            return t[:]

        b_magic = const_col(MAGIC)
        b_negmagic = const_col(-MAGIC)
        b_halfpi = const_col(PI / 2.0)
        b_gamma = const_col(GAMMA)

        # phase = relv * PHASE_SCALE; range-reduce to [-pi, pi]
        phase = pre.tile([P, D2], f32)
        nc.scalar.activation(phase[:], relv[:], AF.Identity, scale=PHASE_SCALE)
        t1 = pre.tile([P, D2], f32)
        nc.scalar.activation(t1[:], phase[:], AF.Identity, scale=INV_TWO_PI, bias=b_magic)
        kf = pre.tile([P, D2], f32)
        nc.scalar.activation(kf[:], t1[:], AF.Identity, bias=b_negmagic)
        ws = pre.tile([P, D2], f32)
        nc.vector.cody_waite_cascade(ws[:], phase[:], kf[:], CW0, CW1, CW2)

        # im_rel = sin(ws); re_rel = cos(ws) = sin(pi/2 - |ws|)
        im_rel = pre.tile([P, D2], f32)
        nc.scalar.activation(im_rel[:], ws[:], AF.Sin)
        aws = pre.tile([P, D2], f32)
        nc.scalar.activation(aws[:], ws[:], AF.Abs)
        re_rel = pre.tile([P, D2], f32)
        nc.scalar.activation(re_rel[:], aws[:], AF.Sin, scale=-1.0, bias=b_halfpi)

        # negrot = -(head rotated):  negrot_re = im_h*im_rel - re_h*re_rel
        #                            negrot_im = -(re_h*im_rel) - im_h*re_rel
        negrot = pre.tile([P, D], f32)
        m_re = pre.tile([P, D2], f32)
        nc.vector.tensor_mul(m_re[:], head_t[:, 0:D2], re_rel[:])
        m_im = pre.tile([P, D2], f32)
        nc.vector.tensor_mul(m_im[:], head_t[:, D2:D], im_rel[:])
        nc.vector.tensor_sub(negrot[:, 0:D2], m_im[:], m_re[:])
        m2 = pre.tile([P, D2], f32)
        nc.vector.tensor_mul(m2[:], head_t[:, 0:D2], im_rel[:])
        m3 = pre.tile([P, D2], f32)
        nc.vector.tensor_mul(m3[:], head_t[:, D2:D], re_rel[:])
        nc.vector.scalar_tensor_tensor(
            negrot[:, D2:D], m2[:], -1.0, m3[:], op0=ALU.mult, op1=ALU.subtract
        )

        ident = const.tile([P, P], f32)
        make_identity(nc, ident[:])
        score_sb = const.tile([P, NEG], f32)

        # ---------- main loop over neg samples ----------
        for j in range(NEG):
            tj = tails.tile([P, D], f32, tag="tj")
            nc.vector.tensor_copy(tj[:], negrot[:])
            nc.gpsimd.indirect_dma_start(
                out=tj[:], out_offset=None, in_=ent[:],
                in_offset=IndirectOffsetOnAxis(ap=tp_t[:, j : j + 1], axis=0),
                compute_op=ALU.add,
            )  # tj = tail - rot
            sq_re = sqp.tile([P, D2], f32, tag="sq_re")
            nc.scalar.activation(sq_re[:], tj[:, 0:D2], AF.Square)
            sq_im = sqp.tile([P, D2], f32, tag="sq_im")
            nc.vector.tensor_mul(sq_im[:], tj[:, D2:D], tj[:, D2:D])
            ps = psum.tile([P, D2], f32, tag="ps")
            nc.tensor.matmul(out=ps[:], lhsT=ident[:], rhs=sq_re[:], start=True, stop=False)
            nc.tensor.matmul(out=ps[:], lhsT=ident[:], rhs=sq_im[:], start=False, stop=True)
            srt = psc.tile([P, D2], f32, tag="srt")
            nc.scalar.activation(
                srt[:], ps[:], AF.Sqrt, accum_out=score_sb[:, j : j + 1]
            )

        # ---------- finale: score = GAMMA - colsum ----------
        out_t = const.tile([P, NEG], f32)
        nc.scalar.activation(out_t[:], score_sb[:], AF.Identity, scale=-1.0, bias=b_gamma)
        nc.sync.dma_start(out=score[:], in_=out_t[:])

    nc.compile()
    return nc


def _get_nc():
    global _CACHED_NC
    if _CACHED_NC is None:
        _CACHED_NC = _build_nc()
    return _CACHED_NC


def _run(inputs, **spmd_kwargs):
    hp = np.ascontiguousarray(np.asarray(inputs["head_part"], dtype=np.int32))
    tp = np.ascontiguousarray(np.asarray(inputs["tail_part"], dtype=np.int32))
    rel = np.ascontiguousarray(np.asarray(inputs["relation_embedding"], dtype=np.float32))
    ent = np.ascontiguousarray(np.asarray(inputs["entity_embedding"], dtype=np.float32))

    in_maps = []
    for c in range(N_CORES):
        sl = slice(c * B_LOC, (c + 1) * B_LOC)
        in_maps.append(
            {
                "head_part": hp[sl],
                "tail_part": tp[sl],
                "relation_embedding": rel,
                "entity_embedding": ent,
            }
        )
    res = run_bass_kernel_spmd(_get_nc(), in_maps, core_ids=list(range(N_CORES)), **spmd_kwargs)
    out = np.concatenate([r["score"] for r in res.results], axis=0)
    return out, res


def kernel(**inputs) -> np.ndarray:
    return _run(inputs)[0]


def kernel_traced(**inputs):
    """Like kernel() but returns (output, BassKernelResults) with HW profile."""
    return _run(inputs, trace=True)


# revision 4
# speedup vs baseline: 1.5341x; 1.5341x over previous
"""RotatE KGE scoring kernel for Trainium2 (Bass/Tile), 8-core data parallel.

Problem (per reference):
  head  = entity_embedding[head_part[:,0]]           # [B,1,1000]
  rel   = relation_embedding[head_part[:,1]]         # [B,1,500]
  tail  = entity_embedding[tail_part]                # [B,128,1000]
  phase = rel / (EMB_RANGE/PI); rot = head * e^{i*phase}  (complex, D/2=500)
  score = GAMMA - sum_d sqrt((rot_re-tail_re)^2 + (rot_im-tail_im)^2)

Sharding: batch dim (1024) split across 8 cores, 128 batches each; embedding
tables replicated. Per core the dominant cost is the tail gather:
128x128 rows x 4KB = 65.5 MB of HBM traffic (memory-bound regime).

Per-core pipeline (j = neg-sample index, 128 iterations):
  Pool: plain indirect row-gather of 128 entity rows -> tile [128,1000]
        (plain, NOT accumulate: CCE-add halves SDMA packet rate and slows
        descriptor generation; measured 321ns/4000B vs 302ns/8000B packets)
  DVE : custom SQDIFF op (out = (in0-in1)^2, registered at import) on each
        half vs the precomputed rot -> bf16 [128,500] tiles
  PE  : bf16 identity-matmul accumulate re^2+im^2 -> f32 PSUM
        (f32 matmul is ~3x slower on PE; bf16 sq values only carry ~2^-9
        relative noise into a 500-term sum -> ~1e-4 on the score)
  ACT : Sqrt with accum_out -> one column of the [128,128] score tile
Final: score = GAMMA - colsums, one activation + one DMA out.
"""

import math
from contextlib import ExitStack

import numpy as np

import concourse.bacc as bacc
import concourse.mybir as mybir
import concourse.tile as tile
from concourse.bass import IndirectOffsetOnAxis
from concourse.bass_utils import run_bass_kernel_spmd
from concourse.masks import make_identity

# ---- problem constants (hardcoded per contract) ----
N_CORES = 8
B = 1024
B_LOC = B // N_CORES  # 128
NEG = 128
N_ENT = 100000
N_REL = 500
D = 1000
D2 = D // 2  # 500

GAMMA = 12.0
EPSILON = 2.0
EMB_RANGE = (GAMMA + EPSILON) / D2  # 0.028
PI = 3.141592653589793
PHASE_SCALE = float(1.0 / (EMB_RANGE / PI))  # multiply instead of divide

TWO_PI = 2.0 * math.pi
INV_TWO_PI = 1.0 / TWO_PI
MAGIC = 1.5 * 2.0**23  # round-to-nearest via fp32 quantization
# Cody-Waite split of 2*pi: c0 exact in fp32, c1 fp32, c2 the f64 remainder
CW0 = 6.28125
CW1 = float(np.float32(TWO_PI - CW0))
CW2 = float(TWO_PI - CW0 - np.float64(np.float32(TWO_PI - CW0)))

f32 = mybir.dt.float32
bf16 = mybir.dt.bfloat16
i32 = mybir.dt.int32
AF = mybir.ActivationFunctionType
ALU = mybir.AluOpType

_CACHED_NC = None
_SQDIFF_OP = None


def _register_sqdiff():
    """Register a custom DVE op computing out = (in0 - in1)^2 in one pass.

    The op registry is a module-level list; codegen and table-gen both read
    it within this process, and the per-NEFF DVE table is generated from it
    at compile time, so a runtime-registered op works exactly like a
    built-in one. The uops sha is computed here (it pins the lowered table
    bytes; we derive it from the actual lowering rather than hardcoding).
    """
    global _SQDIFF_OP
    if _SQDIFF_OP is not None:
        return _SQDIFF_OP
    import concourse.dve_ops as dve_ops
    from concourse.dve_spec import Spec, Src0, Src1, sq, lower, _has_src1
    from concourse.dve_uop import DveOpSpec

    name = "SQDIFF_KGE"
    if name in dve_ops._SUB_OPCODE_FOR_NAME:
        _SQDIFF_OP = next(op for op in dve_ops.OPS if op.name == name)
        return _SQDIFF_OP

    spec = Spec(
        body=sq(Src0 - Src1),
        reference=lambda in0, in1, s0, s1, imm2: (
            (in0.astype(np.float32) - in1.astype(np.float32)) ** 2
        ),
    )
    opcode = dve_ops._CUSTOM_DVE_ROW_BASE + len(dve_ops.OPS)
    assert opcode < 0x20
    shas = {}
    for ver in ("v3", "v4"):
        try:
            uops = lower(spec, ver=ver)
            shas[ver] = DveOpSpec(
                name=name, opcode=opcode, uops=uops, rd1_en=_has_src1(spec)
            ).sha(ver)
        except Exception:
            pass
    op = dve_ops.DveOp(name, spec, subdim=False, uops_sha=shas)
    dve_ops.OPS.append(op)
    dve_ops._SUB_OPCODE_FOR_NAME[name] = opcode
    dve_ops.CUSTOM_DVE_SPECS[name] = spec
    _SQDIFF_OP = op
    return op


def _build_nc():
    sqdiff = _register_sqdiff()
    nc = bacc.Bacc("TRN2", target_bir_lowering=False, debug=False)

    hp = nc.dram_tensor("head_part", [B_LOC, 3], i32, kind="ExternalInput")
    tp = nc.dram_tensor("tail_part", [B_LOC, NEG], i32, kind="ExternalInput")
    rel = nc.dram_tensor("relation_embedding", [N_REL, D2], f32, kind="ExternalInput")
    ent = nc.dram_tensor("entity_embedding", [N_ENT, D], f32, kind="ExternalInput")
    score = nc.dram_tensor("score", [B_LOC, NEG], f32, kind="ExternalOutput")

    P = 128

    with tile.TileContext(nc) as tc, ExitStack() as ctx:
        const = ctx.enter_context(tc.tile_pool(name="const", bufs=1))
        pre = ctx.enter_context(tc.tile_pool(name="pre", bufs=1))
        tails = ctx.enter_context(tc.tile_pool(name="tails", bufs=10))
        sqp = ctx.enter_context(tc.tile_pool(name="sqp", bufs=4))
        psum = ctx.enter_context(tc.tile_pool(name="psum", bufs=4, space="PSUM"))
        psc = ctx.enter_context(tc.tile_pool(name="psc", bufs=2, space="PSUM"))

        # ---------- preamble ----------
        hp_t = const.tile([P, 3], i32)
        nc.sync.dma_start(out=hp_t[:], in_=hp[:])
        tp_t = const.tile([P, NEG], i32)
        nc.sync.dma_start(out=tp_t[:], in_=tp[:])

        head_t = pre.tile([P, D], f32)
        nc.gpsimd.indirect_dma_start(
            out=head_t[:], out_offset=None, in_=ent[:],
            in_offset=IndirectOffsetOnAxis(ap=hp_t[:, 0:1], axis=0),
        )
        relv = pre.tile([P, D2], f32)
        nc.gpsimd.indirect_dma_start(
            out=relv[:], out_offset=None, in_=rel[:],
            in_offset=IndirectOffsetOnAxis(ap=hp_t[:, 1:2], axis=0),
        )

        def const_col(val):
            t = const.tile([P, 1], f32, tag=f"c{val}")
            nc.gpsimd.memset(t[:], float(val))
            return t[:]

        b_magic = const_col(MAGIC)
        b_negmagic = const_col(-MAGIC)
        b_halfpi = const_col(PI / 2.0)
        b_gamma = const_col(GAMMA)

        # phase = relv * PHASE_SCALE; range-reduce to [-pi, pi]
        phase = pre.tile([P, D2], f32)
        nc.scalar.activation(phase[:], relv[:], AF.Identity, scale=PHASE_SCALE)
        t1 = pre.tile([P, D2], f32)
        nc.scalar.activation(t1[:], phase[:], AF.Identity, scale=INV_TWO_PI, bias=b_magic)
        kf = pre.tile([P, D2], f32)
        nc.scalar.activation(kf[:], t1[:], AF.Identity, bias=b_negmagic)
        ws = pre.tile([P, D2], f32)
        nc.vector.cody_waite_cascade(ws[:], phase[:], kf[:], CW0, CW1, CW2)

        # im_rel = sin(ws); re_rel = cos(ws) = sin(pi/2 - |ws|)
        im_rel = pre.tile([P, D2], f32)
        nc.scalar.activation(im_rel[:], ws[:], AF.Sin)
        aws = pre.tile([P, D2], f32)
        nc.scalar.activation(aws[:], ws[:], AF.Abs)
        re_rel = pre.tile([P, D2], f32)
        nc.scalar.activation(re_rel[:], aws[:], AF.Sin, scale=-1.0, bias=b_halfpi)

        # rot_cat = head rotated: rot_re = re_h*re_rel - im_h*im_rel
        #                         rot_im = re_h*im_rel + im_h*re_rel
        rot = pre.tile([P, D], f32)
        m_re = pre.tile([P, D2], f32)
        nc.vector.tensor_mul(m_re[:], head_t[:, 0:D2], re_rel[:])
        m_im = pre.tile([P, D2], f32)
        nc.vector.tensor_mul(m_im[:], head_t[:, D2:D], im_rel[:])
        nc.vector.tensor_sub(rot[:, 0:D2], m_re[:], m_im[:])
        m2 = pre.tile([P, D2], f32)
        nc.vector.tensor_mul(m2[:], head_t[:, 0:D2], im_rel[:])
        m3 = pre.tile([P, D2], f32)
        nc.vector.tensor_mul(m3[:], head_t[:, D2:D], re_rel[:])
        nc.vector.tensor_add(rot[:, D2:D], m2[:], m3[:])

        ident = const.tile([P, P], bf16)
        make_identity(nc, ident[:])
        score_sb = const.tile([P, NEG], f32)

        # ---------- main loop over neg samples ----------
        for j in range(NEG):
            tj = tails.tile([P, D], f32, tag="tj")
            nc.gpsimd.indirect_dma_start(
                out=tj[:], out_offset=None, in_=ent[:],
                in_offset=IndirectOffsetOnAxis(ap=tp_t[:, j : j + 1], axis=0),
            )
            sq_re = sqp.tile([P, D2], bf16, tag="sq_re")
            nc.vector._custom_dve(
                sqdiff, out=sq_re[:], in0=tj[:, 0:D2], in1=rot[:, 0:D2]
            )
            sq_im = sqp.tile([P, D2], bf16, tag="sq_im")
            nc.vector._custom_dve(
                sqdiff, out=sq_im[:], in0=tj[:, D2:D], in1=rot[:, D2:D]
            )
            ps = psum.tile([P, D2], f32, tag="ps")
            nc.tensor.matmul(out=ps[:], lhsT=ident[:], rhs=sq_re[:], start=True, stop=False)
            nc.tensor.matmul(out=ps[:], lhsT=ident[:], rhs=sq_im[:], start=False, stop=True)
            srt = psc.tile([P, D2], f32, tag="srt")
            nc.scalar.activation(
                srt[:], ps[:], AF.Sqrt, accum_out=score_sb[:, j : j + 1]
            )

        # ---------- finale: score = GAMMA - colsum ----------
        out_t = const.tile([P, NEG], f32)
        nc.scalar.activation(out_t[:], score_sb[:], AF.Identity, scale=-1.0, bias=b_gamma)
        nc.sync.dma_start(out=score[:], in_=out_t[:])

    nc.compile()
    return nc


def _get_nc():
    global _CACHED_NC
    if _CACHED_NC is None:
        _CACHED_NC = _build_nc()
    return _CACHED_NC


def _run(inputs, **spmd_kwargs):
    hp = np.ascontiguousarray(np.asarray(inputs["head_part"], dtype=np.int32))
    tp = np.ascontiguousarray(np.asarray(inputs["tail_part"], dtype=np.int32))
    rel = np.ascontiguousarray(np.asarray(inputs["relation_embedding"], dtype=np.float32))
    ent = np.ascontiguousarray(np.asarray(inputs["entity_embedding"], dtype=np.float32))

    in_maps = []
    for c in range(N_CORES):
        sl = slice(c * B_LOC, (c + 1) * B_LOC)
        in_maps.append(
            {
                "head_part": hp[sl],
                "tail_part": tp[sl],
                "relation_embedding": rel,
                "entity_embedding": ent,
            }
        )
    res = run_bass_kernel_spmd(_get_nc(), in_maps, core_ids=list(range(N_CORES)), **spmd_kwargs)
    out = np.concatenate([r["score"] for r in res.results], axis=0)
    return out, res


def kernel(**inputs) -> np.ndarray:
    return _run(inputs)[0]


def kernel_traced(**inputs):
    """Like kernel() but returns (output, BassKernelResults) with HW profile."""
    return _run(inputs, trace=True)
